# revision 25
# baseline (speedup 1.0000x reference)
"""Trainium2 Bass kernel for nn_LLAConv2d: per-sample 1x1 conv with mixed kernels.

Math: out[b,o,h,w] = sum_i K[b,o,i] * x[b,i,h,w],  K[b] = sum_e alpha[b,e]*ke[e]
i.e. a per-sample 64x64 matmul over 160*160=25600 pixels. Memory-bound:
per core (4 samples, bf16 I/O) 13.1 MB in + 13.1 MB out at ~350-390 GB/s
-> ~70-75 us HBM floor; measured ~76-90 us incl. ~8 us NEFF boot.

Strategy (8 cores, data-parallel over batch, 4 samples/core):
  - Pack 2 samples per matmul: block-diagonal lhsT [128,128] (two 64x64 K^T
    blocks), rhs = x tile [128(2*64 chans), N].
  - Kernel mixing on device: one small bf16 preamble DMA ships a pre-zeroed
    block-diag embed table ketd[128, E*128] + per-pair per-partition alpha
    columns; mix with 1 tensor_scalar_mul + 7 scalar_tensor_tensor MACs on
    DVE, cast to bf16 lhsT — all hidden under the first x tile's DMA.
  - Main loop: x tiles of 5120 px (1.31 MB DMA on the sync HWDGE ring,
    7-deep prefetch to ride out HBM contention); PSUM chunks of 1024 px
    (2 banks, 4-deep pool); 2 matmuls of 512 per chunk; PSUM->SBUF cast
    alternates DVE / ACT so neither engine gates the DMA stream; per-tile
    out DMA on the scalar HWDGE ring. Small first tile (fast pipeline fill)
    and small trailing tiles (short drain; last two stores ride the by-then
    idle sync ring).
  - Dispatch via bass2jax.fast_dispatch_compile (C++ fast path); kernel()
    runs the NEFF until two runs agree bit-exactly (masks rare one-off
    DMA/readback corruption).
"""

import sys

sys.path.insert(0, "/opt/trn_rl_repo")

import numpy as np

import concourse.bacc as bacc
import concourse.mybir as mybir
import concourse.tile as tile

F32 = mybir.dt.float32
BF16 = mybir.dt.bfloat16

N_CORES = 8
B, E, CIN, COUT, H, W = 32, 8, 64, 64, 160, 160
PIX = H * W                     # 25600
BPC = B // N_CORES              # 4 samples per core
NPAIR = BPC // 2                # 2 sample-pairs per core

MM_N = 512                      # matmul moving free dim (ISA cap)
CHUNK = 1024                    # PSUM chunk: 2 banks of f32
TILE_N = 5120                   # pixels per DMA tile (1.31 MB bf16)
NT = PIX // TILE_N              # tiles per pair
KPC = TILE_N // CHUNK           # chunks per tile

LAST_RESULTS = None  # optionally set by test.py's trace run

_COMPILED = None  # cache of compiled executable + metadata


def _np_bf16():
    import ml_dtypes

    return np.dtype(ml_dtypes.bfloat16)


def _build_bass():
    nc = bacc.Bacc(trn_type="TRN2", target_bir_lowering=False, debug=False)

    x_d = nc.dram_tensor("x", [NPAIR, 128, PIX], BF16, kind="ExternalInput").ap()
    # wtab = [ketd (E*128) | al (NPAIR*E)] so the whole preamble is one DMA;
    # bf16 so it lands fast ahead of the x stream
    wtab_d = nc.dram_tensor(
        "wtab", [128, E * 128 + NPAIR * E], BF16, kind="ExternalInput"
    ).ap()
    out_d = nc.dram_tensor("out", [NPAIR, 128, PIX], BF16, kind="ExternalOutput").ap()

    MUL = mybir.AluOpType.mult
    ADD = mybir.AluOpType.add

    # per-pair tile sizes (each a CHUNK multiple): pair 0 starts with a small
    # tile so the first casts fire right after the weights are mixed; both
    # pairs end with small tiles to shorten the kernel's drain tail
    tiles_by_pair = [
        [2 * CHUNK, 5120, 5120, 5120, 5120, 2 * CHUNK, CHUNK],
        [5120, 5120, 5120, 5120, 3 * CHUNK, 2 * CHUNK],
    ]
    for tl in tiles_by_pair:
        assert sum(tl) == PIX and all(t % CHUNK == 0 for t in tl)

    with tile.TileContext(nc) as tc:
        with (
            tc.tile_pool(name="wpool", bufs=1) as wpool,
            tc.tile_pool(name="xpool", bufs=7) as xpool,
            tc.tile_pool(name="opool", bufs=7) as opool,
            tc.tile_pool(name="ppool", bufs=4, space="PSUM") as ppool,
        ):
            # --- preamble: one small bf16 DMA, first in the sync ring's FIFO
            # so it lands ~1us ahead of x tile 0
            wtab_sb = wpool.tile([128, E * 128 + NPAIR * E], BF16, tag="wtab_sb")
            nc.sync.dma_start(out=wtab_sb, in_=wtab_d)
            ketd_sb = wtab_sb[:, 0 : E * 128]
            # tensor_scalar needs f32 scalars: one tiny cast of the al columns
            al_sb = wpool.tile([128, NPAIR * E], F32, tag="al_sb")
            nc.vector.tensor_copy(al_sb, wtab_sb[:, E * 128 : E * 128 + NPAIR * E])

            # --- mix weights on DVE: acc = sum_e al[:,e] * ketd[:, e*128:...]
            # (ketd blocks carry the block-diagonal zeros)
            lhsT = []
            for p in range(NPAIR):
                acc = wpool.tile([128, 128], F32, tag=f"acc{p}", name=f"acc{p}")
                nc.vector.tensor_scalar_mul(
                    acc, ketd_sb[:, 0:128], al_sb[:, p * E : p * E + 1]
                )
                for e in range(1, E):
                    nc.vector.scalar_tensor_tensor(
                        acc,
                        ketd_sb[:, e * 128 : (e + 1) * 128],
                        al_sb[:, p * E + e : p * E + e + 1],
                        acc,
                        MUL,
                        ADD,
                    )
                tr = wpool.tile([128, 128], BF16, tag=f"lhsT{p}", name=f"lhsT{p}")
                nc.vector.tensor_copy(tr, acc)
                lhsT.append(tr)

            # --- main loop: x tile in (sync ring) -> matmuls into 2-bank PSUM
            # chunks -> cast PSUM->SBUF alternating DVE/ACT -> out tile (scalar
            # ring)
            cast_i = 0
            for p in range(NPAIR):
                tiles = tiles_by_pair[p]
                c0 = 0
                for jt, tn in enumerate(tiles):
                    xt = xpool.tile([128, tn], BF16, tag="xt")
                    nc.sync.dma_start(out=xt, in_=x_d[p, :, c0 : c0 + tn])
                    ot = opool.tile([128, tn], BF16, tag="ot")
                    for k in range(tn // CHUNK):
                        pt = ppool.tile([128, CHUNK], F32, tag="pt")
                        for h in range(CHUNK // MM_N):
                            s = k * CHUNK + h * MM_N
                            nc.tensor.matmul(
                                pt[:, h * MM_N : (h + 1) * MM_N],
                                lhsT[p],
                                xt[:, s : s + MM_N],
                                start=True,
                                stop=True,
                            )
                        dst = ot[:, k * CHUNK : (k + 1) * CHUNK]
                        if cast_i % 2 == 0:
                            nc.vector.tensor_copy(dst, pt)
                        else:
                            nc.scalar.copy(dst, pt)
                        cast_i += 1
                    # tail tiles of the last pair store via the sync ring —
                    # the in-stream is done by then and the scalar ring would
                    # serialize these behind earlier stores
                    last_tail = p == NPAIR - 1 and jt >= len(tiles) - 2
                    oeng = nc.sync if last_tail else nc.scalar
                    oeng.dma_start(out=out_d[p, :, c0 : c0 + tn], in_=ot)
                    c0 += tn

    nc.compile()
    return nc


def _prep_inputs(x, alpha, kernel_embed):
    x = np.ascontiguousarray(x, dtype=np.float32)
    alpha = np.ascontiguousarray(alpha, dtype=np.float32)
    ke = np.ascontiguousarray(kernel_embed, dtype=np.float32)[:, :, :, 0, 0]

    # ketd[i, e*128+o]: block-diagonal embed table, keT[e][i, o] = ke[e, o, i]
    keT = np.transpose(ke, (0, 2, 1))  # [E, cin, cout]
    ketd = np.zeros((128, E * 128), dtype=np.float32)
    for e in range(E):
        ketd[0:64, e * 128 : e * 128 + 64] = keT[e]
        ketd[64:128, e * 128 + 64 : e * 128 + 128] = keT[e]
    ketd = np.ascontiguousarray(ketd)

    xio = x.astype(_np_bf16())

    in_maps = []
    for c in range(N_CORES):
        xs = xio[c * BPC : (c + 1) * BPC].reshape(NPAIR, 128, PIX)
        als = alpha[c * BPC : (c + 1) * BPC]  # [4, E]
        # al[i, p*E + e] = alpha[2p + (i >= 64), e]
        al = np.repeat(als.reshape(NPAIR, 2, E), 64, axis=1)  # [NPAIR, 128, E]
        al = np.transpose(al, (1, 0, 2)).reshape(128, NPAIR * E)
        wtab = np.concatenate([ketd, al.astype(np.float32)], axis=1)
        in_maps.append(
            {
                "x": np.ascontiguousarray(xs),
                "wtab": np.ascontiguousarray(wtab.astype(_np_bf16())),
            }
        )
    return in_maps


def _get_compiled():
    """Build + compile the sharded fast-dispatch executable once."""
    global _COMPILED
    if _COMPILED is not None:
        return _COMPILED

    import jax
    from jax.sharding import Mesh, NamedSharding, PartitionSpec
    from jax.experimental.shard_map import shard_map

    from concourse import bass2jax

    nc = _build_bass()
    bass2jax.install_neuronx_cc_hook()

    in_names, out_names, out_avals, zero_shapes = [], [], [], []
    for alloc in nc.m.functions[0].allocations:
        if not isinstance(alloc, mybir.MemoryLocationSet):
            continue
        name = alloc.memorylocations[0].name
        pid = nc.partition_id_tensor.name if nc.partition_id_tensor else None
        if alloc.kind == "ExternalInput":
            if name != pid:
                in_names.append(name)
        elif alloc.kind == "ExternalOutput":
            out_names.append(name)
            dtype = mybir.dt.np(alloc.dtype)
            out_avals.append(jax.core.ShapedArray(tuple(alloc.tensor_shape), dtype))
            zero_shapes.append((tuple(alloc.tensor_shape), np.dtype(dtype)))
    n_params = len(in_names)
    all_names = list(in_names) + out_names
    if nc.partition_id_tensor is not None:
        all_names = all_names + [nc.partition_id_tensor.name]

    def _body(*args):
        operands = list(args)
        if nc.partition_id_tensor is not None:
            operands.append(bass2jax.partition_id_tensor())
        return tuple(
            bass2jax._bass_exec_p.bind(
                *operands,
                out_avals=tuple(out_avals),
                in_names=tuple(all_names),
                out_names=tuple(out_names),
                lowering_input_output_aliases=(),
                sim_require_finite=True,
                sim_require_nnan=True,
                nc=nc,
            )
        )

    devices = jax.devices()[:N_CORES]
    mesh = Mesh(np.asarray(devices), ("core",))
    spec = PartitionSpec("core")
    sh = NamedSharding(mesh, spec)
    donate = tuple(range(n_params, n_params + len(out_names)))

    def _jit():
        return jax.jit(
            shard_map(
                _body,
                mesh=mesh,
                in_specs=(spec,) * (n_params + len(out_names)),
                out_specs=(spec,) * len(out_names),
                check_rep=False,
            ),
            donate_argnums=donate,
            keep_unused=True,
        )

    aval_by_name = {
        "x": jax.ShapeDtypeStruct((N_CORES * NPAIR, 128, PIX), _np_bf16(), sharding=sh),
        "wtab": jax.ShapeDtypeStruct(
            (N_CORES * 128, E * 128 + NPAIR * E), _np_bf16(), sharding=sh
        ),
    }
    in_avals = [aval_by_name[n] for n in in_names]
    out_zero_avals = [
        jax.ShapeDtypeStruct((N_CORES * s[0], *s[1:]), d, sharding=sh)
        for s, d in zero_shapes
    ]

    try:
        fn = bass2jax.fast_dispatch_compile(
            lambda: _jit().lower(*in_avals, *out_zero_avals).compile()
        )
    except Exception:
        fn = _jit()  # fall back to the effectful path

    _COMPILED = {
        "fn": fn,
        "nc": nc,
        "in_names": in_names,
        "out_names": out_names,
        "zero_shapes": zero_shapes,
        "sh": sh,
        "n_params": n_params,
        "zero_outs": lambda: [
            jax.device_put(np.zeros((N_CORES * s[0], *s[1:]), d), sh)
            for s, d in zero_shapes
        ],
    }
    return _COMPILED


def _stage_inputs(in_maps, comp):
    import jax

    return [
        jax.device_put(
            np.concatenate([in_maps[c][n] for c in range(N_CORES)], axis=0),
            comp["sh"],
        )
        for n in comp["in_names"]
    ]


def kernel(x, alpha, kernel_embed):
    in_maps = _prep_inputs(x, alpha, kernel_embed)
    comp = _get_compiled()
    concat_in = _stage_inputs(in_maps, comp)

    # Self-checking execution: the NEFF is deterministic, so two clean runs
    # produce bit-identical bf16 outputs. Accept the first value seen twice;
    # this masks rare one-off DMA/readback corruption (seen ~1/10 runs on
    # the first execution of a fresh NEFF). Transient tunnel errors are
    # retried the same way.
    seen = {}
    out_flat = None
    good = None
    last_exc = None
    for _ in range(6):
        try:
            outs = comp["fn"](*concat_in, *comp["zero_outs"]())
            raw = np.asarray(outs[0])  # [N_CORES*NPAIR, 128, PIX] bf16
        except Exception as e:  # transient PJRT/tunnel failure: retry
            last_exc = e
            continue
        good = raw
        key = hash(raw.tobytes())
        if key in seen:
            out_flat = raw
            break
        seen[key] = raw
    if out_flat is None:
        if good is None:
            raise last_exc  # every attempt failed
        out_flat = good  # no two runs agreed; take the last (best effort)
    out_flat = out_flat.astype(np.float32)

    out = np.empty((B, COUT, H, W), dtype=np.float32)
    for c in range(N_CORES):
        out[c * BPC : (c + 1) * BPC] = out_flat[
            c * NPAIR : (c + 1) * NPAIR
        ].reshape(BPC, COUT, H, W)
    return out


def bench(x, alpha, kernel_embed, iters=30):
    """Pipelined per-call wall time (upper bound incl. PJRT dispatch) — only a
    fallback when NTFF profiling is unavailable; see test.py."""
    import time

    import jax

    in_maps = _prep_inputs(x, alpha, kernel_embed)
    comp = _get_compiled()
    fn = comp["fn"]
    concat_in = _stage_inputs(in_maps, comp)

    jax.block_until_ready(fn(*concat_in, *comp["zero_outs"]()))

    calls_per_window = max(50, iters)
    n_windows = 3
    means = []
    for _ in range(n_windows):
        zsets = [comp["zero_outs"]() for _ in range(calls_per_window)]
        jax.block_until_ready(zsets)
        t0 = time.perf_counter()
        outs = [fn(*concat_in, *zs) for zs in zsets]
        jax.block_until_ready(outs)
        means.append((time.perf_counter() - t0) / calls_per_window)
        del outs, zsets

    means_ns = sorted(m * 1e9 for m in means)
    return {
        "serial_min_ns": means_ns[0],
        "serial_med_ns": means_ns[len(means_ns) // 2],
        "pipelined_ns": means_ns[0],
    }


# revision 27
# speedup vs baseline: 1.0449x; 1.0449x over previous
"""Trainium2 Bass kernel for nn_LLAConv2d: per-sample 1x1 conv with mixed kernels.

Math: out[b,o,h,w] = sum_i K[b,o,i] * x[b,i,h,w],  K[b] = sum_e alpha[b,e]*ke[e]
i.e. a per-sample 64x64 matmul over 160*160=25600 pixels. Memory-bound:
per core (4 samples, bf16 I/O) 13.1 MB in + 13.1 MB out at ~350-390 GB/s
-> ~70-75 us HBM floor; measured ~76-90 us incl. ~8 us NEFF boot.

Strategy (8 cores, data-parallel over batch, 4 samples/core):
  - Pack 2 samples per matmul: block-diagonal lhsT [128,128] (two 64x64 K^T
    blocks), rhs = x tile [128(2*64 chans), N].
  - Kernel mixing on device: one small bf16 preamble DMA ships a pre-zeroed
    block-diag embed table ketd[128, E*128] + per-pair per-partition alpha
    columns; mix with 1 tensor_scalar_mul + 7 scalar_tensor_tensor MACs on
    DVE, cast to bf16 lhsT — all hidden under the first x tile's DMA.
  - Main loop: x tiles of 5120 px (1.31 MB DMA on the sync HWDGE ring,
    7-deep prefetch to ride out HBM contention); PSUM chunks of 1024 px
    (2 banks, 4-deep pool); 2 matmuls of 512 per chunk; PSUM->SBUF cast
    alternates DVE / ACT so neither engine gates the DMA stream; per-tile
    out DMA on the scalar HWDGE ring. Small first tile (fast pipeline fill)
    and small trailing tiles (short drain; last two stores ride the by-then
    idle sync ring).
  - Dispatch via bass2jax.fast_dispatch_compile (C++ fast path); kernel()
    runs the NEFF until two runs agree bit-exactly (masks rare one-off
    DMA/readback corruption).
"""

import sys

sys.path.insert(0, "/opt/trn_rl_repo")

import numpy as np

import concourse.bacc as bacc
import concourse.mybir as mybir
import concourse.tile as tile

F32 = mybir.dt.float32
BF16 = mybir.dt.bfloat16

N_CORES = 8
B, E, CIN, COUT, H, W = 32, 8, 64, 64, 160, 160
PIX = H * W                     # 25600
BPC = B // N_CORES              # 4 samples per core
NPAIR = BPC // 2                # 2 sample-pairs per core

MM_N = 512                      # matmul moving free dim (ISA cap)
CHUNK = 1024                    # PSUM chunk: 2 banks of f32
TILE_N = 5120                   # pixels per DMA tile (1.31 MB bf16)
NT = PIX // TILE_N              # tiles per pair
KPC = TILE_N // CHUNK           # chunks per tile

LAST_RESULTS = None  # optionally set by test.py's trace run

_COMPILED = None  # cache of compiled executable + metadata


def _np_bf16():
    import ml_dtypes

    return np.dtype(ml_dtypes.bfloat16)


def _build_bass():
    nc = bacc.Bacc(trn_type="TRN2", target_bir_lowering=False, debug=False)

    x_d = nc.dram_tensor("x", [NPAIR, 128, PIX], BF16, kind="ExternalInput").ap()
    # wtab = [ketd (E*128) | al (NPAIR*E)] so the whole preamble is one DMA;
    # bf16 so it lands fast ahead of the x stream
    wtab_d = nc.dram_tensor(
        "wtab", [128, E * 128 + NPAIR * E], BF16, kind="ExternalInput"
    ).ap()
    out_d = nc.dram_tensor("out", [NPAIR, 128, PIX], BF16, kind="ExternalOutput").ap()

    MUL = mybir.AluOpType.mult
    ADD = mybir.AluOpType.add

    # per-pair tile sizes (each a CHUNK multiple): pair 0 starts with a small
    # tile so the first casts fire right after the weights are mixed; both
    # pairs end with small tiles to shorten the kernel's drain tail
    tiles_by_pair = [
        [2 * CHUNK, 5120, 5120, 5120, 5120, 2 * CHUNK, CHUNK],
        [5120, 5120, 5120, 5120, 3 * CHUNK, 2 * CHUNK],
    ]
    for tl in tiles_by_pair:
        assert sum(tl) == PIX and all(t % CHUNK == 0 for t in tl)

    with tile.TileContext(nc) as tc:
        with (
            tc.tile_pool(name="wpool", bufs=1) as wpool,
            tc.tile_pool(name="xpool", bufs=7) as xpool,
            tc.tile_pool(name="opool", bufs=7) as opool,
            tc.tile_pool(name="ppool", bufs=4, space="PSUM") as ppool,
        ):
            # --- preamble: one small bf16 DMA, first in the sync ring's FIFO
            # so it lands ~1us ahead of x tile 0
            wtab_sb = wpool.tile([128, E * 128 + NPAIR * E], BF16, tag="wtab_sb")
            nc.sync.dma_start(out=wtab_sb, in_=wtab_d)
            ketd_sb = wtab_sb[:, 0 : E * 128]
            # tensor_scalar needs f32 scalars: one tiny cast of the al columns
            al_sb = wpool.tile([128, NPAIR * E], F32, tag="al_sb")
            nc.vector.tensor_copy(al_sb, wtab_sb[:, E * 128 : E * 128 + NPAIR * E])

            # --- mix weights on DVE: acc = sum_e al[:,e] * ketd[:, e*128:...]
            # (ketd blocks carry the block-diagonal zeros)
            lhsT = []
            for p in range(NPAIR):
                acc = wpool.tile([128, 128], F32, tag=f"acc{p}", name=f"acc{p}")
                nc.vector.tensor_scalar_mul(
                    acc, ketd_sb[:, 0:128], al_sb[:, p * E : p * E + 1]
                )
                for e in range(1, E):
                    nc.vector.scalar_tensor_tensor(
                        acc,
                        ketd_sb[:, e * 128 : (e + 1) * 128],
                        al_sb[:, p * E + e : p * E + e + 1],
                        acc,
                        MUL,
                        ADD,
                    )
                tr = wpool.tile([128, 128], BF16, tag=f"lhsT{p}", name=f"lhsT{p}")
                nc.vector.tensor_copy(tr, acc)
                lhsT.append(tr)

            # --- main loop: x tile in (sync ring) -> matmuls into 2-bank
            # PSUM chunks -> cast PSUM->SBUF alternating DVE/ACT -> out tile
            # (scalar ring; the last pair's tail stores ride the by-then idle
            # sync ring)
            cast_i = 0
            ti = 0
            n_tiles = sum(len(t) for t in tiles_by_pair)
            for p in range(NPAIR):
                tiles = tiles_by_pair[p]
                c0 = 0
                for tn in tiles:
                    ieng = nc.sync
                    oeng = nc.sync if ti >= n_tiles - 2 else nc.scalar
                    ti += 1
                    xt = xpool.tile([128, tn], BF16, tag="xt")
                    ieng.dma_start(out=xt, in_=x_d[p, :, c0 : c0 + tn])
                    ot = opool.tile([128, tn], BF16, tag="ot")
                    for k in range(tn // CHUNK):
                        pt = ppool.tile([128, CHUNK], F32, tag="pt")
                        for h in range(CHUNK // MM_N):
                            s = k * CHUNK + h * MM_N
                            nc.tensor.matmul(
                                pt[:, h * MM_N : (h + 1) * MM_N],
                                lhsT[p],
                                xt[:, s : s + MM_N],
                                start=True,
                                stop=True,
                            )
                        dst = ot[:, k * CHUNK : (k + 1) * CHUNK]
                        if cast_i % 2 == 0:
                            nc.vector.tensor_copy(dst, pt)
                        else:
                            nc.scalar.copy(dst, pt)
                        cast_i += 1
                    oeng.dma_start(out=out_d[p, :, c0 : c0 + tn], in_=ot)
                    c0 += tn

    nc.compile()
    return nc


def _prep_inputs(x, alpha, kernel_embed):
    x = np.ascontiguousarray(x, dtype=np.float32)
    alpha = np.ascontiguousarray(alpha, dtype=np.float32)
    ke = np.ascontiguousarray(kernel_embed, dtype=np.float32)[:, :, :, 0, 0]

    # ketd[i, e*128+o]: block-diagonal embed table, keT[e][i, o] = ke[e, o, i]
    keT = np.transpose(ke, (0, 2, 1))  # [E, cin, cout]
    ketd = np.zeros((128, E * 128), dtype=np.float32)
    for e in range(E):
        ketd[0:64, e * 128 : e * 128 + 64] = keT[e]
        ketd[64:128, e * 128 + 64 : e * 128 + 128] = keT[e]
    ketd = np.ascontiguousarray(ketd)

    xio = x.astype(_np_bf16())

    in_maps = []
    for c in range(N_CORES):
        xs = xio[c * BPC : (c + 1) * BPC].reshape(NPAIR, 128, PIX)
        als = alpha[c * BPC : (c + 1) * BPC]  # [4, E]
        # al[i, p*E + e] = alpha[2p + (i >= 64), e]
        al = np.repeat(als.reshape(NPAIR, 2, E), 64, axis=1)  # [NPAIR, 128, E]
        al = np.transpose(al, (1, 0, 2)).reshape(128, NPAIR * E)
        wtab = np.concatenate([ketd, al.astype(np.float32)], axis=1)
        in_maps.append(
            {
                "x": np.ascontiguousarray(xs),
                "wtab": np.ascontiguousarray(wtab.astype(_np_bf16())),
            }
        )
    return in_maps


def _get_compiled():
    """Build + compile the sharded fast-dispatch executable once."""
    global _COMPILED
    if _COMPILED is not None:
        return _COMPILED

    import jax
    from jax.sharding import Mesh, NamedSharding, PartitionSpec
    from jax.experimental.shard_map import shard_map

    from concourse import bass2jax

    nc = _build_bass()
    bass2jax.install_neuronx_cc_hook()

    in_names, out_names, out_avals, zero_shapes = [], [], [], []
    for alloc in nc.m.functions[0].allocations:
        if not isinstance(alloc, mybir.MemoryLocationSet):
            continue
        name = alloc.memorylocations[0].name
        pid = nc.partition_id_tensor.name if nc.partition_id_tensor else None
        if alloc.kind == "ExternalInput":
            if name != pid:
                in_names.append(name)
        elif alloc.kind == "ExternalOutput":
            out_names.append(name)
            dtype = mybir.dt.np(alloc.dtype)
            out_avals.append(jax.core.ShapedArray(tuple(alloc.tensor_shape), dtype))
            zero_shapes.append((tuple(alloc.tensor_shape), np.dtype(dtype)))
    n_params = len(in_names)
    all_names = list(in_names) + out_names
    if nc.partition_id_tensor is not None:
        all_names = all_names + [nc.partition_id_tensor.name]

    def _body(*args):
        operands = list(args)
        if nc.partition_id_tensor is not None:
            operands.append(bass2jax.partition_id_tensor())
        return tuple(
            bass2jax._bass_exec_p.bind(
                *operands,
                out_avals=tuple(out_avals),
                in_names=tuple(all_names),
                out_names=tuple(out_names),
                lowering_input_output_aliases=(),
                sim_require_finite=True,
                sim_require_nnan=True,
                nc=nc,
            )
        )

    devices = jax.devices()[:N_CORES]
    mesh = Mesh(np.asarray(devices), ("core",))
    spec = PartitionSpec("core")
    sh = NamedSharding(mesh, spec)
    donate = tuple(range(n_params, n_params + len(out_names)))

    def _jit():
        return jax.jit(
            shard_map(
                _body,
                mesh=mesh,
                in_specs=(spec,) * (n_params + len(out_names)),
                out_specs=(spec,) * len(out_names),
                check_rep=False,
            ),
            donate_argnums=donate,
            keep_unused=True,
        )

    aval_by_name = {
        "x": jax.ShapeDtypeStruct((N_CORES * NPAIR, 128, PIX), _np_bf16(), sharding=sh),
        "wtab": jax.ShapeDtypeStruct(
            (N_CORES * 128, E * 128 + NPAIR * E), _np_bf16(), sharding=sh
        ),
    }
    in_avals = [aval_by_name[n] for n in in_names]
    out_zero_avals = [
        jax.ShapeDtypeStruct((N_CORES * s[0], *s[1:]), d, sharding=sh)
        for s, d in zero_shapes
    ]

    try:
        fn = bass2jax.fast_dispatch_compile(
            lambda: _jit().lower(*in_avals, *out_zero_avals).compile()
        )
    except Exception:
        fn = _jit()  # fall back to the effectful path

    _COMPILED = {
        "fn": fn,
        "nc": nc,
        "in_names": in_names,
        "out_names": out_names,
        "zero_shapes": zero_shapes,
        "sh": sh,
        "n_params": n_params,
        "zero_outs": lambda: [
            jax.device_put(np.zeros((N_CORES * s[0], *s[1:]), d), sh)
            for s, d in zero_shapes
        ],
    }
    return _COMPILED


def _stage_inputs(in_maps, comp):
    import jax

    return [
        jax.device_put(
            np.concatenate([in_maps[c][n] for c in range(N_CORES)], axis=0),
            comp["sh"],
        )
        for n in comp["in_names"]
    ]


def kernel(x, alpha, kernel_embed):
    in_maps = _prep_inputs(x, alpha, kernel_embed)
    comp = _get_compiled()
    concat_in = _stage_inputs(in_maps, comp)

    # Self-checking execution: the NEFF is deterministic, so two clean runs
    # produce bit-identical bf16 outputs. Accept the first value seen twice;
    # this masks rare one-off DMA/readback corruption (seen ~1/10 runs on
    # the first execution of a fresh NEFF). Transient tunnel errors are
    # retried the same way.
    seen = {}
    out_flat = None
    good = None
    last_exc = None
    for _ in range(6):
        try:
            outs = comp["fn"](*concat_in, *comp["zero_outs"]())
            raw = np.asarray(outs[0])  # [N_CORES*NPAIR, 128, PIX] bf16
        except Exception as e:  # transient PJRT/tunnel failure: retry
            last_exc = e
            continue
        good = raw
        key = hash(raw.tobytes())
        if key in seen:
            out_flat = raw
            break
        seen[key] = raw
    if out_flat is None:
        if good is None:
            raise last_exc  # every attempt failed
        out_flat = good  # no two runs agreed; take the last (best effort)
    out_flat = out_flat.astype(np.float32)

    out = np.empty((B, COUT, H, W), dtype=np.float32)
    for c in range(N_CORES):
        out[c * BPC : (c + 1) * BPC] = out_flat[
            c * NPAIR : (c + 1) * NPAIR
        ].reshape(BPC, COUT, H, W)
    return out


def bench(x, alpha, kernel_embed, iters=30):
    """Pipelined per-call wall time (upper bound incl. PJRT dispatch) — only a
    fallback when NTFF profiling is unavailable; see test.py."""
    import time

    import jax

    in_maps = _prep_inputs(x, alpha, kernel_embed)
    comp = _get_compiled()
    fn = comp["fn"]
    concat_in = _stage_inputs(in_maps, comp)

    jax.block_until_ready(fn(*concat_in, *comp["zero_outs"]()))

    calls_per_window = max(50, iters)
    n_windows = 3
    means = []
    for _ in range(n_windows):
        zsets = [comp["zero_outs"]() for _ in range(calls_per_window)]
        jax.block_until_ready(zsets)
        t0 = time.perf_counter()
        outs = [fn(*concat_in, *zs) for zs in zsets]
        jax.block_until_ready(outs)
        means.append((time.perf_counter() - t0) / calls_per_window)
        del outs, zsets

    means_ns = sorted(m * 1e9 for m in means)
    return {
        "serial_min_ns": means_ns[0],
        "serial_med_ns": means_ns[len(means_ns) // 2],
        "pipelined_ns": means_ns[0],
    }
